# revision 27
# baseline (speedup 1.0000x reference)
"""Trainium2 Bass kernel for CausalAttentionSortNet bucket-scoring.

Math (see reference): only `k` feeds the output. For each merged batch*head
slice, the cumulative-average of k is sampled at bucket starts (every 128th
row), which reduces to per-chunk sums + a strictly-triangular prefix matmul.
The rest is tiny per-bucket sort projections and a 64x65 masked softmax.

Sharding: data-parallel over the merged (batch*heads)=32 axis across 8 cores,
4 slices per core as 2 pairs; partition=(slice_in_pair, chunk), free=(row, dim)
so every partition's k data is one contiguous 32KB HBM run. Both pairs of each
row-group share one SBUF tile so each fold is a single batched instruction.

`q` (half of all input bytes) is never read by the reference computation, so
it is not even transferred to the device.

Structure (all trace-driven):
- DVE does exactly two fold levels per sub-tile (level 1 fp32->fp16
  pair-major, level 2 fp16 writing a row-major tile); the PE absorbs the
  rest as h1/2 accumulating fp16 matmuls per sub-tile against the scaled
  triangular prefix matrix, into ONE fp32 PSUM bank (PT). This keeps DVE
  well under the DMA stream time (deep DVE-only fold chains left a ~3.4us
  DVE backlog after the stream) and keeps the post-stream PE chain short.
- The F path (raw row-0 of each chunk) stays fp32 end to end: F dominates
  the sort projections and fp16 there measures 4-5e-2 output error vs the
  2e-2 gate (fp16 is fine for the chunk-sum/prefix path: ~3e-3 total).
- Per-pair SKQ and R PSUM banks let pair 1's R/softmax/output DMA overlap
  pair 0's. The R banks hold -R (SQ negated during its PSUM->SBUF copy,
  mask seeded +1e30) so the softmax max-negate folds into reduce_min and
  Exp(scale=-1); the exp row-sum fuses in via accum_out. (tensor_reduce's
  negate=True flag miscomputes on HW - do not use it.)
- No PE warm-up / HAM games: flipping the PE clock gate to 8/8 early
  triggers a cooldown state for the REST of the kernel that derates the
  whole datapath (HW-measured: DMA per-packet rate -29%). The cold 4/8 tail
  costs ~1.3us; the cooldown costs far more.

Stream shape notes (HW-measured): one DMA per sub-tile covers both pairs;
all big DMAs on the one sync queue in this exact order (consts at the head
of the queue waste ~2us of ramp; consts on the scalar queue make SDMA
engine 15 straggle ~4us); 8KB-run leading tiles, tail tiles >=8 rows
(3KB/1KB-run tiles crawl <100GB/s); SWDGE fp32->fp16 cast-DMA runs at ~1/4
line rate - do not use it. Fixed, structure-invariant end costs: last-DMA
completion receipt ~1.4us, output-DMA receipt ~1.3us, the runtime wrapper's
~7us semaphore-clear postamble. Run-to-run exec varies ~41.5-46.5us
bimodally with SPMD neighbor alignment on the shared HBM stacks (fast mode
streams at ~415GB/s, fair-share mode at ~320GB/s).
"""

from contextlib import ExitStack

import numpy as np

import concourse.bacc as bacc
import concourse.mybir as mybir
import concourse.tile as tile
from concourse import bass_utils

# Problem constants (hardcoded per contract; kernel.py must be self-contained).
B, HEADS, BUCKETS, DIM, DIM_SORT, T = 4, 8, 64, 64, 8, 8192
BH = B * HEADS            # 32 merged batch*head slices
NCORES = 8
BHC = BH // NCORES        # 4 slices per core
NPAIR = BHC // 2          # 2 pairs per core
CHUNK = T // BUCKETS      # 128 rows per bucket
NEG = -1.0e30             # softmax mask value (underflows exp to exactly 0)
FP = mybir.dt.float32
BF = mybir.dt.bfloat16
F16 = mybir.dt.float16

# rows-per-sub-tile (per pair). Sum = 128. Leading 32-row tiles give 8KB
# descriptor runs; the tail shrinks so the last sub-tile's fold chain is
# short. Do NOT go below 8 rows: 12/4-row tail tiles (3KB/1KB runs) make
# the last ~1MB crawl at <100GB/s and push the final DMA sems ~2-3us later
# (HW-measured).
ROWS = (32, 32, 32, 16, 8, 8)

TRACE = False  # set by test.py for profiling runs
TRACE_KWARGS = {}  # extra run_bass_kernel_spmd kwargs for profiling runs
LAST_RESULTS = None  # BassKernelResults of the most recent run

_PROG_CACHE = {}

# c16 (fp16) column layout: PT-path constants only — the F path must stay
# fp32 (F is raw k and dominates the sort projections; fp16 there measures
# 4-5e-2 output error vs the 2e-2 gate)
_C16_LMAT = 0
_C16_WQPT = 128
_C16_TOT = 336
# cb (bf16) column layout
_CB_ID = 0
_CB_AMASK = 128
_CB_TOT = 193
# cM (fp32) column layout
_CM_ID = 0
_CM_C64 = 128
_CM_C104 = 544
_CM_MMASK = 800
_CM_SCOL = 865
_CM_TOT = 866


def _build_program(enable_asserts=False):
    assert sum(ROWS) == CHUNK, (ROWS, CHUNK)
    nsub = len(ROWS)

    nc = bacc.Bacc(
        "TRN2",
        target_bir_lowering=False,
        debug=False,
        enable_asserts=enable_asserts,
        num_devices=NCORES,
    )

    def din(name, shape, dt=FP):
        return nc.dram_tensor(name, shape, dt, kind="ExternalInput").ap()

    kin = din("kin", (BHC, T, DIM))
    cM = din("cM", (128, _CM_TOT))
    cb = din("cb", (128, _CB_TOT), BF)
    c16 = din("c16", (128, _C16_TOT), F16)
    # out layout (b, i, pair, col): 520B contiguous per (b, i) partition
    rout = nc.dram_tensor(
        "rout", (2, BUCKETS, NPAIR, BUCKETS + 1), FP, kind="ExternalOutput"
    ).ap()

    Exp = mybir.ActivationFunctionType.Exp
    Copy = mybir.ActivationFunctionType.Copy
    MULT = mybir.AluOpType.mult
    X = mybir.AxisListType.X

    with tile.TileContext(nc) as tc:
        with ExitStack() as ctx:
            singles = ctx.enter_context(tc.tile_pool(name="singles", bufs=1))
            kpool = ctx.enter_context(tc.tile_pool(name="kpool", bufs=1))
            small = ctx.enter_context(tc.tile_pool(name="small", bufs=2))
            pp = ctx.enter_context(tc.tile_pool(name="pp", bufs=1, space="PSUM"))

            # ---- everything streams on the one sync queue: sub-tiles 0-1,
            # then the constants, then the rest (the exact v1/v2 order).
            # Consts at the head of the queue waste ~2us of stream ramp
            # (descriptor-dominated), and consts on the scalar queue make
            # SDMA engine 15 straggle ~4us behind the others (both
            # HW-measured) - this ordering is the best of the three.
            ksrc = kin.rearrange(
                "(p b) (c r) d -> (b c) p r d", p=NPAIR, r=CHUNK
            )
            kts = []
            r0 = 0
            for s, rs in enumerate(ROWS):
                kt = kpool.tile([128, NPAIR, rs, DIM], FP, tag=f"kt{s}")
                nc.sync.dma_start(kt[:], ksrc[:, :, r0 : r0 + rs, :])
                kts.append(kt)
                r0 += rs
                if s == 1:
                    cb_sb = singles.tile([128, _CB_TOT], BF, tag="cb")
                    nc.sync.dma_start(cb_sb[:], cb)
                    c16_sb = singles.tile([128, _C16_TOT], F16, tag="c16")
                    nc.sync.dma_start(c16_sb[:], c16)
                    cM_sb = singles.tile([128, _CM_TOT], FP, tag="cM")
                    nc.sync.dma_start(cM_sb[:], cM)

            lmat_s = c16_sb[:, _C16_LMAT : _C16_LMAT + 128]
            ident_bf = cb_sb[:, _CB_ID : _CB_ID + 128]
            amask = cb_sb[:, _CB_AMASK : _CB_AMASK + 65]
            ident = cM_sb[:, _CM_ID : _CM_ID + 128]
            c64_sb = cM_sb[:, _CM_C64 : _CM_C64 + 416]
            c104_sb = cM_sb[0:104, _CM_C104 : _CM_C104 + 256]
            mmask = cM_sb[:, _CM_MMASK : _CM_MMASK + 65]
            s_col = cM_sb[:, _CM_SCOL : _CM_SCOL + 1]

            # ---- PSUM tiles (8 banks)
            PT_ps = pp.tile([128, 128], FP, tag="PT")
            FT_ps = pp.tile([128, 128], FP, tag="FT")
            FTs_ps = pp.tile([128, 128], FP, tag="FTs")
            SKQs = [
                pp.tile([104, 128], FP, tag=f"SKQ{p}", name=f"SKQ{p}")
                for p in range(NPAIR)
            ]
            Rs = [
                pp.tile([128, BUCKETS + 1], FP, tag=f"R{p}", name=f"R{p}")
                for p in range(NPAIR)
            ]
            dummy = pp.tile([128, 128], FP, tag="dummy")

            # NOTE: no PE warm-up burst. Flipping the HAM clock gate to 8/8
            # early triggers a LONG 4/8 "cooldown" state afterwards that
            # derates the whole datapath (HW-measured: DMA per-packet rate
            # drops ~29% while it is active). The natural fold-matmul cadence
            # flips 8/8 only near stream end, which is where it helps.

            # ---- early PE seeds (consts-gated): R additive-mask seeds +
            # SKQ pos-emb const seeds
            for p in range(NPAIR):
                nc.tensor.matmul(
                    Rs[p][:], lhsT=ident_bf, rhs=amask,
                    start=True, stop=False, skip_group_check=True,
                )
            # ---- per-sub-tile folds. Big tiles: lvl1 contiguous fp32->fp16,
            # lvl2 fp16 row-major, h2 matmuls. Small tiles (h1<=4): lvl1
            # writes row-major directly, h1 matmuls. All accumulate into PT.
            first_mm = [True]

            def fold_subtile(s, rs):
                kt = kts[s]
                h1 = rs // 2
                h2 = h1 // 2
                k16 = kpool.tile(
                    [128, NPAIR, h1, DIM], F16, tag=f"k16_{s}",
                    name=f"k16_{s}",
                )
                nc.vector.tensor_add(
                    k16[:], kt[:, :, 0:h1, :], kt[:, :, h1 : 2 * h1, :]
                )
                kr = kpool.tile(
                    [128, h2, NPAIR, DIM], F16, tag=f"kr{s}", name=f"kr{s}"
                )
                nc.vector.tensor_add(
                    kr.rearrange("p r q d -> p q r d"),
                    k16[:, :, 0:h2, :],
                    k16[:, :, h2 : 2 * h2, :],
                )
                if s == nsub - 1 and h2 == 2:
                    # last (8-row) tile: one more tiny DVE fold level halves
                    # its PE matmuls - the PE chain after the final DMA sem
                    # is the critical path. (Only the LAST tile: the same
                    # fold on the second-to-last tile sits on in-order DVE
                    # AHEAD of the last tile's chain and delays it.)
                    kr3 = kpool.tile(
                        [128, 1, NPAIR, DIM], F16, tag=f"kr3_{s}",
                        name=f"kr3_{s}",
                    )
                    nc.vector.tensor_add(kr3[:, 0], kr[:, 0], kr[:, 1])
                    kr, h2 = kr3, 1
                for r in range(h2):
                    nc.tensor.matmul(
                        PT_ps[:], lhsT=kr[:, r], rhs=lmat_s,
                        start=first_mm[0],
                        stop=s == nsub - 1 and r == h2 - 1,
                        skip_group_check=True,
                    )
                    first_mm[0] = False

            # first two fold groups precede the cM-gated seeds/F-path in PE
            # program order (cM lands ~19.5us, behind 4MB of k)
            fold_subtile(0, ROWS[0])
            fold_subtile(1, ROWS[1])

            for p in range(NPAIR):
                nc.tensor.matmul(
                    SKQs[p][:], lhsT=ident[0:104, 0:104],
                    rhs=c104_sb[:, 128 * p : 128 * p + 128],
                    start=True, stop=False, skip_group_check=True,
                )

            # ---- F path (fp32: F is raw k and dominates the projections):
            # row 0 of sub-tile 0, contiguous copies, Fs = F * s[c] via ACT
            # copy with per-partition scale, PE transposes, 4 projections.
            kt0 = kts[0]
            F_sb = small.tile([128, NPAIR, DIM], FP, tag="F")
            nc.vector.tensor_copy(F_sb[:], kt0[:, :, 0, :])
            Fs_sb = small.tile([128, NPAIR, DIM], FP, tag="Fs")
            nc.scalar.activation(Fs_sb[:], kt0[:, :, 0, :], Copy, scale=s_col)
            nc.tensor.matmul(
                FT_ps[:], lhsT=F_sb[:], rhs=ident, start=True, stop=True,
                skip_group_check=True,
            )
            nc.tensor.matmul(
                FTs_ps[:], lhsT=Fs_sb[:], rhs=ident, start=True, stop=True
            )
            FT_sb = small.tile([128, 128], FP, tag="FTsb")
            nc.scalar.copy(FT_sb[:], FT_ps[:])
            FTs_sb = small.tile([128, 128], FP, tag="FTssb")
            nc.scalar.copy(FTs_sb[:], FTs_ps[:])
            for p in range(NPAIR):
                prow = slice(64 * p, 64 * p + 64)
                # wq2/wk2 @ F^T  (raw F part of the sort projections)
                nc.tensor.matmul(
                    SKQs[p][:],
                    lhsT=c64_sb[prow, 208 + 104 * p : 208 + 104 * p + 104],
                    rhs=FT_sb[prow, :],
                    start=False, stop=False, skip_group_check=True,
                )
                # wq1/wk1 @ (s*F)^T  (bucket 0's cumavg IS F: own s*F term)
                nc.tensor.matmul(
                    SKQs[p][:],
                    lhsT=c64_sb[prow, 104 * p : 104 * p + 104],
                    rhs=FTs_sb[prow, :],
                    start=False, stop=False, skip_group_check=True,
                )

            for s in range(2, nsub):
                fold_subtile(s, ROWS[s])

            # ---- tail: PT -> fp16 -> per-pair sort projections -> R ->
            # softmax -> out, pair 1 first throughout so its output DMA is in
            # flight while pair 0 is still in R/softmax. The cast is split by
            # partition half so pair 1's projection starts one half earlier.
            PT_sb = small.tile([128, 128], F16, tag="PTs")
            for p in (1, 0):
                prow = slice(64 * p, 64 * p + 64)
                nc.vector.tensor_copy(PT_sb[prow, :], PT_ps[prow, :])
                nc.tensor.matmul(
                    SKQs[p][:],
                    lhsT=c16_sb[prow, _C16_WQPT + 104 * p : _C16_WQPT + 104 * p + 104],
                    rhs=PT_sb[prow, :],
                    start=False, stop=True, skip_group_check=True,
                )
            # SQ on scalar, RK on vector: the two copies of each pair
            # overlap. SQ is negated during the copy so the R banks hold
            # -R: the softmax max-negate then folds into reduce_min + the
            # Exp's scale=-1 (the reduce negate flag miscomputes on HW).
            SQs = [None, None]
            RKs = [None, None]
            for p in (1, 0):
                sq_sb = small.tile([40, 128], FP, tag=f"SQ{p}", name=f"SQ{p}")
                nc.scalar.activation(
                    sq_sb[:], SKQs[p][0:40, :], Copy, scale=-1.0
                )
                rk_sb = small.tile([40, 128], FP, tag=f"RK{p}", name=f"RK{p}")
                nc.vector.tensor_copy(rk_sb[:], SKQs[p][64:104, :])
                SQs[p] = sq_sb
                RKs[p] = rk_sb
            for p in (1, 0):
                nc.tensor.matmul(
                    Rs[p][0:64, 1:],
                    lhsT=SQs[p][0:8, 0:64],
                    rhs=RKs[p][0:8, 0:64],
                    start=False, stop=False, skip_group_check=True,
                )
                nc.tensor.matmul(
                    Rs[p][64:128, 1:],
                    lhsT=SQs[p][32:40, 64:128],
                    rhs=RKs[p][32:40, 64:128],
                    start=False, stop=True, skip_group_check=True,
                )

            # masked softmax over 65 logits: banks hold -R (zero-logit col 0
            # explicit from the seed), so exp(R-max) = Exp(scale=-1,
            # bias=min(-R)) and the sum fuses in via accum_out.
            negm = small.tile([128, NPAIR], FP, tag="negm")
            e_sb = small.tile([128, NPAIR, BUCKETS + 1], FP, tag="e")
            s1 = small.tile([128, NPAIR], FP, tag="s1")
            rin = small.tile([128, NPAIR], FP, tag="rin")
            outt = small.tile([128, NPAIR, BUCKETS + 1], FP, tag="outt")
            for p in (1, 0):
                nc.vector.tensor_reduce(
                    negm[:, p : p + 1], Rs[p][:],
                    op=mybir.AluOpType.min, axis=X,
                )
                nc.scalar.activation(
                    e_sb[:, p, :], Rs[p][:], Exp,
                    bias=negm[:, p : p + 1], scale=-1.0,
                    accum_out=s1[:, p : p + 1],
                )
                nc.vector.reciprocal(rin[:, p : p + 1], s1[:, p : p + 1])
                # outt = (e * 1/den) * tril-mask, fused
                nc.vector.scalar_tensor_tensor(
                    outt[:, p, :],
                    e_sb[:, p, :],
                    rin[:, p : p + 1],
                    mmask,
                    op0=MULT,
                    op1=MULT,
                )
                dst = rout[:, :, p, :].rearrange("b i c -> (b i) c")
                if p == 1:
                    nc.sync.dma_start(dst, outt[:, p, :])
                else:
                    nc.scalar.dma_start(dst, outt[:, p, :])

    nc.compile()
    return nc


def _get_program(enable_asserts=False):
    key = enable_asserts
    if key not in _PROG_CACHE:
        _PROG_CACHE[key] = _build_program(enable_asserts=enable_asserts)
    return _PROG_CACHE[key]


def _host_constants(core, q_pos_emb, k_pos_emb, Wsq, Wsk):
    """Tiny per-core packed constant tensors."""
    import ml_dtypes

    f32 = np.float32
    f16 = np.float16
    j = np.arange(64, dtype=np.float64)
    s = (1.0 / (CHUNK * j + 1.0)).astype(f32)  # per-bucket cumavg scale

    tri = np.triu(np.ones((64, 64), f32), k=1)  # [c, j] = 1 iff c < j
    tri_s = tri * s[None, :]
    lmat_s = np.zeros((128, 128), f32)
    lmat_s[0:64, 0:64] = tri_s
    lmat_s[64:128, 64:128] = tri_s

    q = np.arange(64)[:, None]
    jc = np.arange(65)[None, :]
    # R banks hold -R, so masked logits seed +1e30 (Exp scale=-1 -> -inf)
    am = np.where(jc > q, -NEG, 0.0).astype(f32)  # softmax mask, col0 free
    mm = (jc < q).astype(f32)                     # output tril(-1) mask incl col0
    amask = np.concatenate([am, am], axis=0)      # (128, 65) both b blocks
    mmask = np.concatenate([mm, mm], axis=0)

    wq_pt = np.zeros((2, 64, 104), f32)   # [pair][d][sq 0:40 | sk 64:104]
    wq_ft = np.zeros((2, 64, 104), f32)
    cblk = np.zeros((2, 104, 128), f32)   # [pair][skq-row][(b, j)]
    for p in range(NPAIR):
        for b in range(2):
            bh = core * BHC + 2 * p + b
            h = bh % HEADS
            r0 = 32 * b
            wq_pt[p, :, r0 : r0 + 8] = Wsq[0, h, 0:64, :]
            wq_pt[p, :, 64 + r0 : 64 + r0 + 8] = Wsk[0, h, 0:64, :]
            wq_ft[p, :, r0 : r0 + 8] = Wsq[0, h, 64:128, :]
            wq_ft[p, :, 64 + r0 : 64 + r0 + 8] = Wsk[0, h, 64:128, :]
            cq = q_pos_emb[0, h] @ Wsq[0, h, 128:192, :]  # (64, 8)
            ck = k_pos_emb[0, h] @ Wsk[0, h, 128:192, :]
            cblk[p, r0 : r0 + 8, 64 * b : 64 * b + 64] = cq.T
            cblk[p, 64 + r0 : 64 + r0 + 8, 64 * b : 64 * b + 64] = ck.T

    def dup(a):  # duplicate into both b halves along rows
        return np.concatenate([a, a], axis=0)

    wq_pt2 = dup(np.concatenate([wq_pt[0], wq_pt[1]], axis=1))   # (128, 208)
    c64 = dup(np.concatenate([wq_pt[0], wq_pt[1], wq_ft[0], wq_ft[1]], axis=1))
    c104p = np.zeros((128, 256), f32)
    c104p[0:104] = np.concatenate([cblk[0], cblk[1]], axis=1)
    ident = np.eye(128, dtype=f32)

    c16 = np.concatenate([lmat_s, wq_pt2], axis=1).astype(f16)
    assert c16.shape == (128, _C16_TOT), c16.shape

    cbm = np.concatenate(
        [ident, amask], axis=1
    ).astype(ml_dtypes.bfloat16)
    assert cbm.shape == (128, _CB_TOT), cbm.shape

    s_col = np.concatenate([s, s])[:, None]   # (128, 1) per-partition scale
    cMm = np.concatenate([ident, c64, c104p, mmask, s_col], axis=1)
    assert cMm.shape == (128, _CM_TOT), cMm.shape

    return {"cM": cMm, "cb": cbm, "c16": c16}


def _run(k, q_pos_emb, k_pos_emb, Wsq, Wsk, trace=False):
    nc = _get_program()
    in_maps = []
    for core in range(NCORES):
        cm = _host_constants(core, q_pos_emb, k_pos_emb, Wsq, Wsk)
        cm["kin"] = np.ascontiguousarray(k[core * BHC : (core + 1) * BHC])
        in_maps.append(cm)
    res = bass_utils.run_bass_kernel_spmd(
        nc,
        in_maps,
        core_ids=list(range(NCORES)),
        trace=trace,
        **(TRACE_KWARGS if trace else {}),
    )
    global LAST_RESULTS
    LAST_RESULTS = res
    out = np.empty((BH, BUCKETS, BUCKETS + 1), np.float32)
    for core, r in enumerate(res.results):
        ro = r["rout"]  # (2, 64, 2, 65) = (b, i, pair, col)
        for p in range(NPAIR):
            for b in range(2):
                out[core * BHC + 2 * p + b] = ro[b, :, p, :]
    return out, res


def kernel(**inputs):
    k = np.asarray(inputs["k"], np.float32)
    q_pos_emb = np.asarray(inputs["q_pos_emb"], np.float32)
    k_pos_emb = np.asarray(inputs["k_pos_emb"], np.float32)
    Wsq = np.asarray(inputs["Wsq"], np.float32)
    Wsk = np.asarray(inputs["Wsk"], np.float32)
    out, _ = _run(k, q_pos_emb, k_pos_emb, Wsq, Wsk, trace=TRACE)
    return out
